# revision 1
# baseline (speedup 1.0000x reference)
"""IsoGMM loss kernel for 8 Trainium2 NeuronCores.

loss = mean_{n,k} r[n,k] * ||X[n] - mus[k]||^2

Decomposition (the entire loss folds into ONE accumulated PE matmul per core):
  sum_{n,k} r*d2 = T1 + T2 - 2*T3
    T1 = sum_n xsq_n * R_n        (xsq_n = ||X[n]||^2, R_n = sum_k r[n,k])
    T2 = sum_k musq_k * C_k       (C_k = sum_n r[n,k])
    T3 = sum_{k,d} mus[k,d] * M[k,d],  M = r.T @ X

Host augments X rows to width 130: [X | 1 | xsq-slot]; xsq is filled
on-chip (ACT square + DVE per-row reduce). Per 128-row segment:
  ps[64,130] += r_seg.T @ [X | 1 | xsq]_seg
giving cols 0:128 = M, col 128 = C_k, col 129 = A_k = sum_n r[n,k]*xsq_n
(T1 = sum_k A_k). Final partial = sum([-2*mus | musq | 1] * ps).

Sharding: data-parallel over N, 16384 rows per core. Each SBUF partition
holds 128 *contiguous* rows (row order is irrelevant for every term), so
every DMA is perfectly contiguous per partition.
"""

import numpy as np

import concourse.bass as bass
import concourse.mybir as mybir
import concourse.tile as tile
from concourse import bacc
from concourse.bass_utils import run_bass_kernel_spmd

N, K, D = 131072, 64, 128
NCORES = 8
W = D + 2            # augmented row width: 128 data + ones + xsq slot
NS = N // NCORES     # rows per core
RPP = NS // 128      # rows per SBUF partition (= segments per core)
CHUNKS = 16          # DMA/compute pipeline chunks per core


def build_nc(rpp=RPP, chunks=CHUNKS):
    segs = rpp
    spc = segs // chunks       # segments per chunk
    assert spc * chunks == segs
    xf = rpp * W
    rf = rpp * K
    f32 = mybir.dt.float32

    # Bacc (not plain Bass): its compile() splits sync waits to satisfy
    # TRN2's 1-wait-per-instruction limit, which walrus enforces.
    nc = bacc.Bacc("TRN2", target_bir_lowering=False, debug=False)
    xp = nc.dram_tensor("xp", [128, xf], f32, kind="ExternalInput")
    rp = nc.dram_tensor("rp", [128, rf], f32, kind="ExternalInput")
    out = nc.dram_tensor("out", [K, W], f32, kind="ExternalOutput")

    with (
        tile.TileContext(nc) as tc,
        tc.tile_pool(name="xb", bufs=3) as xpool,
        tc.tile_pool(name="rb", bufs=3) as rpool,
        tc.tile_pool(name="scr", bufs=2) as spool,
        tc.tile_pool(name="one", bufs=1) as onepool,
        tc.tile_pool(name="ps", bufs=1, space="PSUM") as pspool,
    ):
        ps = pspool.tile([K, W], f32)

        for c in range(chunks):
            xt = xpool.tile([128, spc * W], f32, tag="x")
            rt = rpool.tile([128, spc * K], f32, tag="r")
            nc.sync.dma_start(out=xt, in_=xp[:, c * spc * W:(c + 1) * spc * W])
            nc.sync.dma_start(out=rt, in_=rp[:, c * spc * K:(c + 1) * spc * K])

            x3 = xt.rearrange("p (s w) -> p s w", w=W)
            r3 = rt.rearrange("p (s k) -> p s k", k=K)

            # per-row ||x||^2: DVE squares the chunk (table-based ACT
            # functions fault the exec unit under axon), DVE row-reduces
            # into the xsq slot (col 129 of each augmented row).
            sq = spool.tile([128, spc * D], f32, tag="sq")
            sq3 = sq.rearrange("p (s d) -> p s d", d=D)
            nc.vector.tensor_mul(sq3, x3[:, :, 0:D], x3[:, :, 0:D])
            nc.vector.reduce_sum(
                x3[:, :, D + 1:D + 2], sq3, axis=mybir.AxisListType.X
            )

            for j in range(spc):
                s = c * spc + j
                nc.tensor.matmul(
                    ps,
                    lhsT=r3[:, j, :],
                    rhs=x3[:, j, :],
                    start=(s == 0),
                    stop=(s == segs - 1),
                )

        # Ship the accumulated [K, W] panel; the final 64x130-element
        # weighted sum is part of host-side unsharding.
        osb = onepool.tile([K, W], f32)
        nc.vector.tensor_copy(osb, ps)
        nc.sync.dma_start(out=out[:, :], in_=osb)

    nc.compile()
    return nc


def make_in_maps(X, r, mus, ncores=NCORES):
    X = np.ascontiguousarray(np.asarray(X, dtype=np.float32))
    r = np.ascontiguousarray(np.asarray(r, dtype=np.float32))
    mus = np.ascontiguousarray(np.asarray(mus, dtype=np.float32))
    n = X.shape[0]
    ns = n // ncores

    in_maps = []
    for i in range(ncores):
        Xs = X[i * ns:(i + 1) * ns]
        Xa = np.empty((ns, W), np.float32)
        Xa[:, :D] = Xs
        Xa[:, D] = 1.0
        Xa[:, D + 1] = 0.0
        in_maps.append(
            {
                "xp": np.ascontiguousarray(Xa.reshape(128, (ns // 128) * W)),
                "rp": np.ascontiguousarray(
                    r[i * ns:(i + 1) * ns].reshape(128, (ns // 128) * K)
                ),
            }
        )
    return in_maps


def combine_outputs(results, mus):
    """Unshard: weighted sum of each core's [K, W] panel -> mean."""
    mus = np.asarray(mus, dtype=np.float32)
    musq = (mus.astype(np.float64) ** 2).sum(1)
    ma = np.concatenate(
        [-2.0 * mus.astype(np.float64), musq[:, None], np.ones((K, 1))], axis=1
    )
    total = 0.0
    for res in results:
        total += float((ma * res["out"].astype(np.float64)).sum())
    return np.array(total / (N * K), dtype=np.float32)


def kernel(X, r, mus):
    nc = build_nc()
    in_maps = make_in_maps(X, r, mus)
    res = run_bass_kernel_spmd(nc, in_maps, list(range(NCORES)))
    return combine_outputs(res.results[:NCORES], mus)



# revision 2
# speedup vs baseline: 1.2972x; 1.2972x over previous
"""IsoGMM loss kernel for 8 Trainium2 NeuronCores.

loss = mean_{n,k} r[n,k] * ||X[n] - mus[k]||^2

Decomposition (the entire loss folds into ONE accumulated PE matmul per core):
  sum_{n,k} r*d2 = T1 + T2 - 2*T3
    T1 = sum_n xsq_n * R_n        (xsq_n = ||X[n]||^2, R_n = sum_k r[n,k])
    T2 = sum_k musq_k * C_k       (C_k = sum_n r[n,k])
    T3 = sum_{k,d} mus[k,d] * M[k,d],  M = r.T @ X

Host augments X rows to width 130: [X | 1 | xsq-slot]; xsq is filled
on-chip (GPSIMD square + DVE per-row reduce). Per 128-row segment:
  ps[64,130] += r_seg.T @ [X | 1 | xsq]_seg
giving cols 0:128 = M, col 128 = C_k, col 129 = A_k = sum_n r[n,k]*xsq_n
(T1 = sum_k A_k). Final partial = sum([-2*mus | musq | 1] * ps).

X and r are shipped in bf16 (the 2e-2 rel-err budget dwarfs bf16 noise,
measured ~5e-5): halves HBM traffic and runs the PE at full bf16 rate
instead of the 4x-slower fp32 path.

Sharding: data-parallel over N, 16384 rows per core. Each SBUF partition
holds 128 *contiguous* rows (row order is irrelevant for every term), so
every DMA is perfectly contiguous per partition.
"""

import numpy as np
import ml_dtypes

import concourse.bass as bass
import concourse.mybir as mybir
import concourse.tile as tile
from concourse import bacc
from concourse.bass_utils import run_bass_kernel_spmd

N, K, D = 131072, 64, 128
NCORES = 8
W = D + 2            # augmented row width: 128 data + ones + xsq slot
NS = N // NCORES     # rows per core
RPP = NS // 128      # rows per SBUF partition (= segments per core)
CHUNKS = 8           # DMA/compute pipeline chunks per core

BF16 = mybir.dt.bfloat16
NP_BF16 = ml_dtypes.bfloat16


def build_nc(rpp=RPP, chunks=CHUNKS):
    segs = rpp
    spc = segs // chunks       # segments per chunk
    assert spc * chunks == segs
    xf = rpp * W
    rf = rpp * K
    f32 = mybir.dt.float32

    # Bacc (not plain Bass): its compile() splits sync waits to satisfy
    # TRN2's 1-wait-per-instruction limit, which walrus enforces.
    nc = bacc.Bacc("TRN2", target_bir_lowering=False, debug=False)
    xp = nc.dram_tensor("xp", [128, xf], BF16, kind="ExternalInput")
    rp = nc.dram_tensor("rp", [128, rf], BF16, kind="ExternalInput")
    out = nc.dram_tensor("out", [K, W], f32, kind="ExternalOutput")

    with (
        tile.TileContext(nc) as tc,
        tc.tile_pool(name="xb", bufs=3) as xpool,
        tc.tile_pool(name="rb", bufs=3) as rpool,
        tc.tile_pool(name="scr", bufs=2) as spool,
        tc.tile_pool(name="one", bufs=1) as onepool,
        tc.tile_pool(name="ps", bufs=1, space="PSUM") as pspool,
    ):
        ps = pspool.tile([K, W], f32)

        for c in range(chunks):
            xt = xpool.tile([128, spc * W], BF16, tag="x")
            rt = rpool.tile([128, spc * K], BF16, tag="r")
            nc.sync.dma_start(out=xt, in_=xp[:, c * spc * W:(c + 1) * spc * W])
            nc.sync.dma_start(out=rt, in_=rp[:, c * spc * K:(c + 1) * spc * K])

            x3 = xt.rearrange("p (s w) -> p s w", w=W)
            r3 = rt.rearrange("p (s k) -> p s k", k=K)

            # per-row ||x||^2: GPSIMD squares the chunk (keeps DVE free
            # for the reduce; table-based ACT functions fault under axon),
            # DVE row-reduces into the xsq slot (col 129 of each row).
            sq = spool.tile([128, spc * D], BF16, tag="sq")
            sq3 = sq.rearrange("p (s d) -> p s d", d=D)
            nc.gpsimd.tensor_mul(sq3, x3[:, :, 0:D], x3[:, :, 0:D])
            with nc.allow_low_precision(reason="xsq noise ~1e-4 of loss"):
                nc.vector.reduce_sum(
                    x3[:, :, D + 1:D + 2], sq3, axis=mybir.AxisListType.X
                )

            for j in range(spc):
                s = c * spc + j
                nc.tensor.matmul(
                    ps,
                    lhsT=r3[:, j, :],
                    rhs=x3[:, j, :],
                    start=(s == 0),
                    stop=(s == segs - 1),
                )

        # Ship the accumulated [K, W] panel; the final 64x130-element
        # weighted sum is part of host-side unsharding.
        osb = onepool.tile([K, W], f32)
        nc.vector.tensor_copy(osb, ps)
        nc.sync.dma_start(out=out[:, :], in_=osb)

    nc.compile()
    return nc


def make_in_maps(X, r, mus, ncores=NCORES):
    X = np.ascontiguousarray(np.asarray(X, dtype=np.float32))
    r = np.ascontiguousarray(np.asarray(r, dtype=np.float32))
    n = X.shape[0]
    ns = n // ncores

    Xb = X.astype(NP_BF16)
    rb = r.astype(NP_BF16)

    in_maps = []
    for i in range(ncores):
        Xa = np.empty((ns, W), NP_BF16)
        Xa[:, :D] = Xb[i * ns:(i + 1) * ns]
        Xa[:, D] = 1.0
        Xa[:, D + 1] = 0.0
        in_maps.append(
            {
                "xp": np.ascontiguousarray(Xa.reshape(128, (ns // 128) * W)),
                "rp": np.ascontiguousarray(
                    rb[i * ns:(i + 1) * ns].reshape(128, (ns // 128) * K)
                ),
            }
        )
    return in_maps


def combine_outputs(results, mus):
    """Unshard: weighted sum of each core's [K, W] panel -> mean."""
    mus = np.asarray(mus, dtype=np.float32)
    musq = (mus.astype(np.float64) ** 2).sum(1)
    ma = np.concatenate(
        [-2.0 * mus.astype(np.float64), musq[:, None], np.ones((K, 1))], axis=1
    )
    total = 0.0
    for res in results:
        total += float((ma * res["out"].astype(np.float64)).sum())
    return np.array(total / (N * K), dtype=np.float32)


def kernel(X, r, mus):
    nc = build_nc()
    in_maps = make_in_maps(X, r, mus)
    res = run_bass_kernel_spmd(nc, in_maps, list(range(NCORES)))
    return combine_outputs(res.results[:NCORES], mus)
